# revision 3
# baseline (speedup 1.0000x reference)
import sys

sys.path.insert(0, "/opt/trn_rl_repo")

import numpy as np
from concourse import bass, mybir, bacc
import concourse.tile as tile
from concourse.bass_utils import run_bass_kernel_spmd
from concourse.masks import make_identity

T, B, OBS, H, C, A = 512, 64, 128, 1024, 512, 32
N_CORES = 8
B_LOC = B // N_CORES
F32 = mybir.dt.float32
I32 = mybir.dt.int32

_CACHE = {}
_LAST_NC = None


def _schedule(hm):
    m = np.asarray(hm).reshape(T, B)
    wave = np.zeros((T, B), np.int32)
    wave[0] = np.where(m[0] == 1, 0, 1)
    for t in range(1, T):
        wave[t] = np.where(m[t] == 1, 0, wave[t - 1] + 1)
    K = int(wave.max()) + 1
    counts = np.zeros((N_CORES, K), np.int64)
    for c in range(N_CORES):
        counts[c] = np.bincount(
            wave[:, c * B_LOC : (c + 1) * B_LOC].ravel(), minlength=K
        )
    chunks = np.maximum(np.ceil(counts.max(axis=0) / 128).astype(np.int64), 1)
    offs = np.zeros(K + 1, np.int64)
    offs[1:] = np.cumsum(chunks * 128)
    R = int(offs[K])
    G = int(chunks[1:].sum()) if K > 1 else 0
    return wave, K, chunks, offs, R, G


def _build_nc(K, chunks, offs, R, G):
    nc = bacc.Bacc(None, target_bir_lowering=False)
    Gp = max(G, 1)
    xsT = nc.declare_dram_parameter("xsT", [128, R], F32, isOutput=False)
    idxp = nc.declare_dram_parameter("idxp", [128, Gp], I32, isOutput=False)
    h0p = nc.declare_dram_parameter("h0p", [B_LOC, C], F32, isOutput=False)
    c0p = nc.declare_dram_parameter("c0p", [B_LOC, C], F32, isOutput=False)
    WinT = nc.declare_dram_parameter("WinT", [128, H], F32, isOutput=False)
    WihT = nc.declare_dram_parameter("WihT", [128, 16384], F32, isOutput=False)
    WhhT = nc.declare_dram_parameter("WhhT", [128, 8192], F32, isOutput=False)
    WmT = nc.declare_dram_parameter("WmT", [128, 4 * A], F32, isOutput=False)
    Hst = nc.declare_dram_parameter("Hst", [R + B_LOC, C], F32, isOutput=True)
    Cst = nc.declare_dram_parameter("Cst", [R + B_LOC, C], F32, isOutput=True)
    Mout = nc.declare_dram_parameter("Mout", [R, A], F32, isOutput=True)

    AFT = mybir.ActivationFunctionType
    with tile.TileContext(nc) as tc:
        with tc.tile_pool(name="w", bufs=1) as wp, \
             tc.tile_pool(name="s", bufs=2) as sp, \
             tc.tile_pool(name="p", bufs=1, space="PSUM") as pp:
            w_in = wp.tile([128, H], F32)
            w_ih = wp.tile([128, 16384], F32)
            w_hh = wp.tile([128, 8192], F32)
            w_m = wp.tile([128, 4 * A], F32)
            ident = wp.tile([128, 128], F32)
            idx_sb = wp.tile([128, Gp], I32)
            h0_sb = wp.tile([B_LOC, C], F32)
            c0_sb = wp.tile([B_LOC, C], F32)

            nc.sync.dma_start(w_in[:], WinT[:])
            for s in range(8):
                nc.sync.dma_start(
                    w_ih[:, s * 2048 : (s + 1) * 2048],
                    WihT[:, s * 2048 : (s + 1) * 2048],
                )
            for s in range(4):
                nc.sync.dma_start(
                    w_hh[:, s * 2048 : (s + 1) * 2048],
                    WhhT[:, s * 2048 : (s + 1) * 2048],
                )
            nc.sync.dma_start(w_m[:], WmT[:])
            nc.sync.dma_start(idx_sb[:], idxp[:])
            make_identity(nc, ident[:])
            nc.sync.dma_start(h0_sb[:], h0p[:])
            nc.sync.dma_start(c0_sb[:], c0p[:])
            nc.sync.dma_start(Hst[R : R + B_LOC, :], h0_sb[:])
            nc.sync.dma_start(Cst[R : R + B_LOC, :], c0_sb[:])

            gi = 0
            for k in range(K):
                nck = int(chunks[k])
                base = int(offs[k])
                for j in range(nck):
                    off = base + j * 128
                    if k > 0:
                        col = gi + j
                        hp = sp.tile([128, C], F32, tag="hp", bufs=3,
                                     name=f"hp_{k}_{j}")
                        cp = sp.tile([128, C], F32, tag="cp", bufs=3,
                                     name=f"cp_{k}_{j}")
                        nc.gpsimd.indirect_dma_start(
                            out=hp[:], out_offset=None, in_=Hst[:],
                            in_offset=bass.IndirectOffsetOnAxis(
                                ap=idx_sb[:, col : col + 1], axis=0))
                        nc.gpsimd.indirect_dma_start(
                            out=cp[:], out_offset=None, in_=Cst[:],
                            in_offset=bass.IndirectOffsetOnAxis(
                                ap=idx_sb[:, col : col + 1], axis=0))
                    xs_c = sp.tile([128, 128], F32, tag="xs", name=f"xs_{k}_{j}")
                    nc.sync.dma_start(xs_c[:], xsT[:, off : off + 128])
                    px = pp.tile([128, H], F32, tag="x", name=f"px_{k}_{j}")
                    for hs in range(8):
                        nc.tensor.matmul(
                            px[:, hs * 128 : (hs + 1) * 128],
                            lhsT=w_in[:, hs * 128 : (hs + 1) * 128],
                            rhs=xs_c[:], start=True, stop=True)
                    xr = sp.tile([128, H], F32, tag="xr", name=f"xr_{k}_{j}")
                    nc.scalar.activation(xr[:, 0:512], px[:, 0:512], AFT.Relu)
                    nc.scalar.activation(xr[:, 512:1024], px[:, 512:1024], AFT.Relu)

                    if k > 0:
                        ptr = pp.tile([128, C], F32, tag="x", name=f"ptr_{k}_{j}")
                        hpT = sp.tile([128, C], F32, tag="hpT", name=f"hpT_{k}_{j}")
                        for cs in range(4):
                            nc.tensor.transpose(
                                ptr[:, cs * 128 : (cs + 1) * 128],
                                hp[:, cs * 128 : (cs + 1) * 128], ident[:])
                        nc.vector.tensor_copy(hpT[:], ptr[:])

                    pg = pp.tile([128, 4 * C], F32, tag="g", name=f"pg_{k}_{j}")
                    banks = (0, 1, 2, 3) if k > 0 else (0, 2, 3)
                    for n in banks:
                        for hs in range(8):
                            nc.tensor.matmul(
                                pg[:, n * 512 : (n + 1) * 512],
                                lhsT=xr[:, hs * 128 : (hs + 1) * 128],
                                rhs=w_ih[:, hs * 2048 + n * 512 : hs * 2048 + (n + 1) * 512],
                                start=(hs == 0), stop=(k == 0 and hs == 7))
                        if k > 0:
                            for cs in range(4):
                                nc.tensor.matmul(
                                    pg[:, n * 512 : (n + 1) * 512],
                                    lhsT=hpT[:, cs * 128 : (cs + 1) * 128],
                                    rhs=w_hh[:, cs * 2048 + n * 512 : cs * 2048 + (n + 1) * 512],
                                    start=False, stop=(cs == 3))

                    si = sp.tile([128, C], F32, tag="si", name=f"si_{k}_{j}")
                    tg = sp.tile([128, C], F32, tag="tg", name=f"tg_{k}_{j}")
                    so = sp.tile([128, C], F32, tag="so", name=f"so_{k}_{j}")
                    nc.scalar.activation(si[:], pg[:, 0:512], AFT.Sigmoid)
                    nc.scalar.activation(tg[:], pg[:, 1024:1536], AFT.Tanh)
                    nc.scalar.activation(so[:], pg[:, 1536:2048], AFT.Sigmoid)
                    ig = sp.tile([128, C], F32, tag="ig", name=f"ig_{k}_{j}")
                    nc.vector.tensor_mul(ig[:], si[:], tg[:])
                    if k > 0:
                        sf = sp.tile([128, C], F32, tag="sf", name=f"sf_{k}_{j}")
                        nc.scalar.activation(sf[:], pg[:, 512:1024], AFT.Sigmoid)
                        fc = sp.tile([128, C], F32, tag="fc", name=f"fc_{k}_{j}")
                        nc.vector.tensor_mul(fc[:], sf[:], cp[:])
                        cn = sp.tile([128, C], F32, tag="cn", name=f"cn_{k}_{j}")
                        nc.vector.tensor_add(cn[:], ig[:], fc[:])
                    else:
                        cn = ig
                    tcn = sp.tile([128, C], F32, tag="tcn", name=f"tcn_{k}_{j}")
                    nc.scalar.activation(tcn[:], cn[:], AFT.Tanh)
                    hn = sp.tile([128, C], F32, tag="hn", name=f"hn_{k}_{j}")
                    nc.vector.tensor_mul(hn[:], so[:], tcn[:])
                    nc.sync.dma_start(Hst[off : off + 128, :], hn[:])
                    nc.sync.dma_start(Cst[off : off + 128, :], cn[:])

                    ptr2 = pp.tile([128, C], F32, tag="x", name=f"ptr2_{k}_{j}")
                    hnT = sp.tile([128, C], F32, tag="hnT", name=f"hnT_{k}_{j}")
                    for cs in range(4):
                        nc.tensor.transpose(
                            ptr2[:, cs * 128 : (cs + 1) * 128],
                            hn[:, cs * 128 : (cs + 1) * 128], ident[:])
                    nc.vector.tensor_copy(hnT[:], ptr2[:])
                    pm = pp.tile([128, A], F32, tag="x", name=f"pm_{k}_{j}")
                    for cs in range(4):
                        nc.tensor.matmul(
                            pm[:],
                            lhsT=hnT[:, cs * 128 : (cs + 1) * 128],
                            rhs=w_m[:, cs * A : (cs + 1) * A],
                            start=(cs == 0), stop=(cs == 3))
                    mo = sp.tile([128, A], F32, tag="mo", name=f"mo_{k}_{j}")
                    nc.scalar.activation(mo[:], pm[:], AFT.Tanh)
                    nc.sync.dma_start(Mout[off : off + 128, :], mo[:])
                if k > 0:
                    gi += nck
    nc.finalize()
    return nc


def _percore_layout(wave, K, chunks, offs, R, G, core):
    bsl = slice(core * B_LOC, (core + 1) * B_LOC)
    wv = wave[:, bsl].ravel()
    cnt = np.bincount(wv, minlength=K)
    order = np.argsort(wv, kind="stable")
    pos = np.empty(T * B_LOC, np.int64)
    start = 0
    for k in range(K):
        rows_k = order[start : start + cnt[k]]
        pos[rows_k] = offs[k] + np.arange(len(rows_k))
        start += cnt[k]
    idx_cols = np.full((128, max(G, 1)), R, np.int32)
    start = cnt[0]
    gcol = 0
    for k in range(1, K):
        rows_k = order[start : start + cnt[k]]
        start += cnt[k]
        tt = rows_k // B_LOC
        bb = rows_k % B_LOC
        pred = np.where(tt == 0, R + bb, pos[np.maximum(rows_k - B_LOC, 0)])
        arr = np.full(int(chunks[k]) * 128, R, np.int64)
        arr[: len(pred)] = pred
        for j in range(int(chunks[k])):
            idx_cols[:, gcol] = arr[j * 128 : (j + 1) * 128]
            gcol += 1
    return pos, idx_cols


def kernel(**inputs):
    global _LAST_NC
    xs = np.asarray(inputs["xs"], np.float32)
    h0 = np.asarray(inputs["h0"], np.float32)
    c0 = np.asarray(inputs["c0"], np.float32)
    hm = np.asarray(inputs["h_masks"])
    W_in = np.asarray(inputs["W_in"], np.float32)
    W_ih = np.asarray(inputs["W_ih"], np.float32)
    W_hh = np.asarray(inputs["W_hh"], np.float32)
    W_mean = np.asarray(inputs["W_mean"], np.float32)
    log_std_param = np.asarray(inputs["log_std_param"], np.float32)

    key = hm.tobytes()
    if key not in _CACHE:
        wave, K, chunks, offs, R, G = _schedule(hm)
        nc = _build_nc(K, chunks, offs, R, G)
        lay = [_percore_layout(wave, K, chunks, offs, R, G, c) for c in range(N_CORES)]
        _CACHE[key] = (wave, K, chunks, offs, R, G, nc, lay)
    wave, K, chunks, offs, R, G, nc, lay = _CACHE[key]
    _LAST_NC = nc

    WinT = np.ascontiguousarray(W_in.T)
    WihT = np.ascontiguousarray(
        W_ih.T.reshape(8, 128, 4 * C).transpose(1, 0, 2).reshape(128, 16384))
    WhhT = np.ascontiguousarray(
        W_hh.T.reshape(4, 128, 4 * C).transpose(1, 0, 2).reshape(128, 8192))
    WmT = np.ascontiguousarray(
        W_mean.T.reshape(4, 128, A).transpose(1, 0, 2).reshape(128, 4 * A))

    t_ids = np.arange(T * B_LOC) // B_LOC
    b_ids = np.arange(T * B_LOC) % B_LOC
    in_maps = []
    for c in range(N_CORES):
        pos, idx_cols = lay[c]
        xsTc = np.zeros((128, R), np.float32)
        xsTc[:, pos] = xs[t_ids, c * B_LOC + b_ids, :].T
        in_maps.append({
            "xsT": xsTc,
            "idxp": idx_cols,
            "h0p": np.ascontiguousarray(h0[c * B_LOC : (c + 1) * B_LOC]),
            "c0p": np.ascontiguousarray(c0[c * B_LOC : (c + 1) * B_LOC]),
            "WinT": WinT,
            "WihT": WihT,
            "WhhT": WhhT,
            "WmT": WmT,
        })

    res = run_bass_kernel_spmd(nc, in_maps, list(range(N_CORES)), trace=False)

    means = np.empty((T, B, A), np.float32)
    hT = np.empty((B, C), np.float32)
    cT = np.empty((B, C), np.float32)
    last_rids = (T - 1) * B_LOC + np.arange(B_LOC)
    for c in range(N_CORES):
        pos, _ = lay[c]
        bsl = slice(c * B_LOC, (c + 1) * B_LOC)
        means[:, bsl, :] = res.results[c]["Mout"][pos].reshape(T, B_LOC, A)
        hT[bsl] = res.results[c]["Hst"][pos[last_rids]]
        cT[bsl] = res.results[c]["Cst"][pos[last_rids]]
    log_std = np.broadcast_to(log_std_param, means.shape).astype(np.float32)
    return means, log_std, (hT, cT)


# revision 10
# speedup vs baseline: 2.1633x; 2.1633x over previous
import sys

sys.path.insert(0, "/opt/trn_rl_repo")

import numpy as np
import ml_dtypes
from concourse import bass, mybir, bacc
import concourse.tile as tile
from concourse.bass_utils import run_bass_kernel_spmd
from concourse.masks import make_identity

T, B, OBS, H, C, A = 512, 64, 128, 1024, 512, 32
N_CORES = 8
B_LOC = B // N_CORES
F32 = mybir.dt.float32
BF16 = mybir.dt.bfloat16
I32 = mybir.dt.int32

_CACHE = {}
_LAST_NC = None


def _schedule(hm):
    m = np.asarray(hm).reshape(T, B)
    wave = np.zeros((T, B), np.int32)
    wave[0] = np.where(m[0] == 1, 0, 1)
    for t in range(1, T):
        wave[t] = np.where(m[t] == 1, 0, wave[t - 1] + 1)
    K = int(wave.max()) + 1
    counts = np.zeros((N_CORES, K), np.int64)
    for c in range(N_CORES):
        counts[c] = np.bincount(
            wave[:, c * B_LOC : (c + 1) * B_LOC].ravel(), minlength=K
        )
    chunks = np.maximum(np.ceil(counts.max(axis=0) / 128).astype(np.int64), 1)
    offs = np.zeros(K + 1, np.int64)
    offs[1:] = np.cumsum(chunks * 128)
    R = int(offs[K])
    G = int(chunks[1:].sum()) if K > 1 else 0
    return wave, K, chunks, offs, R, G


def _build_nc(K, chunks, offs, R, G):
    nc = bacc.Bacc(None, target_bir_lowering=False)
    Gp = max(G, 1)
    xsT = nc.declare_dram_parameter("xsT", [128, R], F32, isOutput=False)
    idxp = nc.declare_dram_parameter("idxp", [128, Gp], I32, isOutput=False)
    h0p = nc.declare_dram_parameter("h0p", [B_LOC, C], F32, isOutput=False)
    c0p = nc.declare_dram_parameter("c0p", [B_LOC, C], F32, isOutput=False)
    WinT = nc.declare_dram_parameter("WinT", [128, H], F32, isOutput=False)
    WihT = nc.declare_dram_parameter("WihT", [128, 16384], BF16, isOutput=False)
    WhhT = nc.declare_dram_parameter("WhhT", [128, 8192], BF16, isOutput=False)
    WmT = nc.declare_dram_parameter("WmT", [128, 4 * A], F32, isOutput=False)
    Hst = nc.declare_dram_parameter("Hst", [R + B_LOC, C], F32, isOutput=True)
    Cst = nc.declare_dram_parameter("Cst", [R + B_LOC, C], F32, isOutput=True)
    Mout = nc.declare_dram_parameter("Mout", [R, A], F32, isOutput=True)

    AFT = mybir.ActivationFunctionType
    with tile.TileContext(nc) as tc:
        with tc.tile_pool(name="w", bufs=1) as wp, \
             tc.tile_pool(name="s", bufs=2) as sp, \
             tc.tile_pool(name="p", bufs=1, space="PSUM") as pp:
            w_in = wp.tile([128, H], F32)
            w_ih = wp.tile([128, 16384], BF16)
            w_hh = wp.tile([128, 8192], BF16)
            w_m = wp.tile([128, 4 * A], F32)
            ident = wp.tile([128, 128], F32)
            idx_sb = wp.tile([128, Gp], I32)
            h0_sb = wp.tile([B_LOC, C], F32)
            c0_sb = wp.tile([B_LOC, C], F32)

            nc.sync.dma_start(w_in[:], WinT[:])
            for s in range(8):
                nc.sync.dma_start(
                    w_ih[:, s * 2048 : (s + 1) * 2048],
                    WihT[:, s * 2048 : (s + 1) * 2048],
                )
            for s in range(4):
                nc.sync.dma_start(
                    w_hh[:, s * 2048 : (s + 1) * 2048],
                    WhhT[:, s * 2048 : (s + 1) * 2048],
                )
            nc.sync.dma_start(w_m[:], WmT[:])
            nc.sync.dma_start(idx_sb[:], idxp[:])
            make_identity(nc, ident[:])
            nc.sync.dma_start(h0_sb[:], h0p[:])
            nc.sync.dma_start(c0_sb[:], c0p[:])
            nc.sync.dma_start(Hst[R : R + B_LOC, :], h0_sb[:])
            nc.sync.dma_start(Cst[R : R + B_LOC, :], c0_sb[:])

            gi = 0
            for k in range(K):
                nck = int(chunks[k])
                base = int(offs[k])
                for j in range(nck):
                    off = base + j * 128
                    if k > 0:
                        col = gi + j
                        hp = sp.tile([128, C], F32, tag="hp", bufs=3,
                                     name=f"hp_{k}_{j}")
                        cp = sp.tile([128, C], F32, tag="cp", bufs=3,
                                     name=f"cp_{k}_{j}")
                        nc.gpsimd.indirect_dma_start(
                            out=hp[:], out_offset=None, in_=Hst[:],
                            in_offset=bass.IndirectOffsetOnAxis(
                                ap=idx_sb[:, col : col + 1], axis=0))
                        nc.gpsimd.indirect_dma_start(
                            out=cp[:], out_offset=None, in_=Cst[:],
                            in_offset=bass.IndirectOffsetOnAxis(
                                ap=idx_sb[:, col : col + 1], axis=0))
                    xs_c = sp.tile([128, 128], F32, tag="xs", name=f"xs_{k}_{j}")
                    nc.sync.dma_start(xs_c[:], xsT[:, off : off + 128])
                    px = pp.tile([128, H], F32, tag="x", name=f"px_{k}_{j}")
                    for hs in range(8):
                        nc.tensor.matmul(
                            px[:, hs * 128 : (hs + 1) * 128],
                            lhsT=w_in[:, hs * 128 : (hs + 1) * 128],
                            rhs=xs_c[:], start=True, stop=True)
                    xr = sp.tile([128, H], BF16, tag="xr", name=f"xr_{k}_{j}")
                    nc.scalar.activation(xr[:, 0:512], px[:, 0:512], AFT.Relu)
                    nc.scalar.activation(xr[:, 512:1024], px[:, 512:1024], AFT.Relu)

                    if k > 0:
                        ptr = pp.tile([128, C], F32, tag="x", name=f"ptr_{k}_{j}")
                        hpT = sp.tile([128, C], BF16, tag="hpT", name=f"hpT_{k}_{j}")
                        for cs in range(4):
                            nc.tensor.transpose(
                                ptr[:, cs * 128 : (cs + 1) * 128],
                                hp[:, cs * 128 : (cs + 1) * 128], ident[:])
                        nc.vector.tensor_copy(hpT[:], ptr[:])

                    pg = pp.tile([128, 4 * C], F32, tag="g", name=f"pg_{k}_{j}")
                    banks = (0, 1, 2, 3) if k > 0 else (0, 2, 3)
                    for n in banks:
                        for hs in range(8):
                            nc.tensor.matmul(
                                pg[:, n * 512 : (n + 1) * 512],
                                lhsT=xr[:, hs * 128 : (hs + 1) * 128],
                                rhs=w_ih[:, hs * 2048 + n * 512 : hs * 2048 + (n + 1) * 512],
                                start=(hs == 0), stop=(k == 0 and hs == 7))
                        if k > 0:
                            for cs in range(4):
                                nc.tensor.matmul(
                                    pg[:, n * 512 : (n + 1) * 512],
                                    lhsT=hpT[:, cs * 128 : (cs + 1) * 128],
                                    rhs=w_hh[:, cs * 2048 + n * 512 : cs * 2048 + (n + 1) * 512],
                                    start=False, stop=(cs == 3))

                    si = sp.tile([128, C], F32, tag="si", name=f"si_{k}_{j}")
                    tg = sp.tile([128, C], F32, tag="tg", name=f"tg_{k}_{j}")
                    so = sp.tile([128, C], F32, tag="so", name=f"so_{k}_{j}")
                    nc.scalar.activation(si[:], pg[:, 0:512], AFT.Sigmoid)
                    nc.scalar.activation(tg[:], pg[:, 1024:1536], AFT.Tanh)
                    nc.scalar.activation(so[:], pg[:, 1536:2048], AFT.Sigmoid)
                    ig = sp.tile([128, C], F32, tag="ig", name=f"ig_{k}_{j}")
                    nc.vector.tensor_mul(ig[:], si[:], tg[:])
                    if k > 0:
                        sf = sp.tile([128, C], F32, tag="sf", name=f"sf_{k}_{j}")
                        nc.scalar.activation(sf[:], pg[:, 512:1024], AFT.Sigmoid)
                        fc = sp.tile([128, C], F32, tag="fc", name=f"fc_{k}_{j}")
                        nc.vector.tensor_mul(fc[:], sf[:], cp[:])
                        cn = sp.tile([128, C], F32, tag="cn", name=f"cn_{k}_{j}")
                        nc.vector.tensor_add(cn[:], ig[:], fc[:])
                    else:
                        cn = ig
                    tcn = sp.tile([128, C], F32, tag="tcn", name=f"tcn_{k}_{j}")
                    nc.scalar.activation(tcn[:], cn[:], AFT.Tanh)
                    hn = sp.tile([128, C], F32, tag="hn", name=f"hn_{k}_{j}")
                    nc.vector.tensor_mul(hn[:], so[:], tcn[:])
                    nc.sync.dma_start(Hst[off : off + 128, :], hn[:])
                    nc.sync.dma_start(Cst[off : off + 128, :], cn[:])

                    ptr2 = pp.tile([128, C], F32, tag="x", name=f"ptr2_{k}_{j}")
                    hnT = sp.tile([128, C], F32, tag="hnT", name=f"hnT_{k}_{j}")
                    for cs in range(4):
                        nc.tensor.transpose(
                            ptr2[:, cs * 128 : (cs + 1) * 128],
                            hn[:, cs * 128 : (cs + 1) * 128], ident[:])
                    nc.vector.tensor_copy(hnT[:], ptr2[:])
                    pm = pp.tile([128, A], F32, tag="x", name=f"pm_{k}_{j}")
                    for cs in range(4):
                        nc.tensor.matmul(
                            pm[:],
                            lhsT=hnT[:, cs * 128 : (cs + 1) * 128],
                            rhs=w_m[:, cs * A : (cs + 1) * A],
                            start=(cs == 0), stop=(cs == 3))
                    mo = sp.tile([128, A], F32, tag="mo", name=f"mo_{k}_{j}")
                    nc.scalar.activation(mo[:], pm[:], AFT.Tanh)
                    nc.sync.dma_start(Mout[off : off + 128, :], mo[:])
                if k > 0:
                    gi += nck
    nc.finalize()
    return nc


def _percore_layout(wave, K, chunks, offs, R, G, core):
    bsl = slice(core * B_LOC, (core + 1) * B_LOC)
    wv = wave[:, bsl].ravel()
    cnt = np.bincount(wv, minlength=K)
    order = np.argsort(wv, kind="stable")
    pos = np.empty(T * B_LOC, np.int64)
    start = 0
    for k in range(K):
        rows_k = order[start : start + cnt[k]]
        pos[rows_k] = offs[k] + np.arange(len(rows_k))
        start += cnt[k]
    idx_cols = np.full((128, max(G, 1)), R, np.int32)
    start = cnt[0]
    gcol = 0
    for k in range(1, K):
        rows_k = order[start : start + cnt[k]]
        start += cnt[k]
        tt = rows_k // B_LOC
        bb = rows_k % B_LOC
        pred = np.where(tt == 0, R + bb, pos[np.maximum(rows_k - B_LOC, 0)])
        arr = np.full(int(chunks[k]) * 128, R, np.int64)
        arr[: len(pred)] = pred
        for j in range(int(chunks[k])):
            idx_cols[:, gcol] = arr[j * 128 : (j + 1) * 128]
            gcol += 1
    return pos, idx_cols


def kernel(**inputs):
    global _LAST_NC
    xs = np.asarray(inputs["xs"], np.float32)
    h0 = np.asarray(inputs["h0"], np.float32)
    c0 = np.asarray(inputs["c0"], np.float32)
    hm = np.asarray(inputs["h_masks"])
    W_in = np.asarray(inputs["W_in"], np.float32)
    W_ih = np.asarray(inputs["W_ih"], np.float32)
    W_hh = np.asarray(inputs["W_hh"], np.float32)
    W_mean = np.asarray(inputs["W_mean"], np.float32)
    log_std_param = np.asarray(inputs["log_std_param"], np.float32)

    key = hm.tobytes()
    if key not in _CACHE:
        wave, K, chunks, offs, R, G = _schedule(hm)
        nc = _build_nc(K, chunks, offs, R, G)
        lay = [_percore_layout(wave, K, chunks, offs, R, G, c) for c in range(N_CORES)]
        _CACHE[key] = (wave, K, chunks, offs, R, G, nc, lay)
    wave, K, chunks, offs, R, G, nc, lay = _CACHE[key]
    _LAST_NC = nc

    WinT = np.ascontiguousarray(W_in.T)
    WihT = np.ascontiguousarray(
        W_ih.T.reshape(8, 128, 4 * C).transpose(1, 0, 2).reshape(128, 16384)
    ).astype(ml_dtypes.bfloat16)
    WhhT = np.ascontiguousarray(
        W_hh.T.reshape(4, 128, 4 * C).transpose(1, 0, 2).reshape(128, 8192)
    ).astype(ml_dtypes.bfloat16)
    WmT = np.ascontiguousarray(
        W_mean.T.reshape(4, 128, A).transpose(1, 0, 2).reshape(128, 4 * A))

    t_ids = np.arange(T * B_LOC) // B_LOC
    b_ids = np.arange(T * B_LOC) % B_LOC
    in_maps = []
    for c in range(N_CORES):
        pos, idx_cols = lay[c]
        xsTc = np.zeros((128, R), np.float32)
        xsTc[:, pos] = xs[t_ids, c * B_LOC + b_ids, :].T
        in_maps.append({
            "xsT": xsTc,
            "idxp": idx_cols,
            "h0p": np.ascontiguousarray(h0[c * B_LOC : (c + 1) * B_LOC]),
            "c0p": np.ascontiguousarray(c0[c * B_LOC : (c + 1) * B_LOC]),
            "WinT": WinT,
            "WihT": WihT,
            "WhhT": WhhT,
            "WmT": WmT,
        })

    res = run_bass_kernel_spmd(nc, in_maps, list(range(N_CORES)), trace=False)

    means = np.empty((T, B, A), np.float32)
    hT = np.empty((B, C), np.float32)
    cT = np.empty((B, C), np.float32)
    last_rids = (T - 1) * B_LOC + np.arange(B_LOC)
    for c in range(N_CORES):
        pos, _ = lay[c]
        bsl = slice(c * B_LOC, (c + 1) * B_LOC)
        means[:, bsl, :] = res.results[c]["Mout"][pos].reshape(T, B_LOC, A)
        hT[bsl] = res.results[c]["Hst"][pos[last_rids]]
        cT[bsl] = res.results[c]["Cst"][pos[last_rids]]
    log_std = np.broadcast_to(log_std_param, means.shape).astype(np.float32)
    return means, log_std, (hT, cT)
